# revision 13
# baseline (speedup 1.0000x reference)
"""Trainium2 Bass kernel: 3x3 stride-1 pad-1 conv, x(32,3,224,224) * W(64,3,3,3) + b -> (32,64,224,224).

Strategy (per core, data-parallel over batch: 4 images/core on 8 cores):
  - Padded coordinate space: 226x226 per image. An SBUF "shift tile" S holds,
    on 18 partitions (g in {0,1} x dx in {0,1,2} x ci in {0,1,2}), the image
    shifted so that partition (g,dx,ci) at free position p=r*226+c contains
    xpad[ci, r+2g, c+dx].
  - One output chunk = 4 output rows = psum[128, 452]:
      rows 0:64   = channels for output rows h..h+1   (block g=0)
      rows 64:128 = channels for output rows h+2..h+3 (block g=1)
    computed as 3 PSUM-accumulating matmuls (dy=0,1,2) with
      lhsT = Wdy [18,128] block-diagonal, rhs = S[:, (h-h0+dy)*226 : +452].
    float32r dtype -> 1 cycle/row on the PE at N>=256.
  - PSUM -> SBUF staging copy adds bias (DVE tensor_scalar_add / ACT
    activation Identity, alternating), then large strided DMAs write HBM.
"""

import os
import sys

import numpy as np

for _p in ("/opt/trn_rl_repo", "/root/.axon_site/_ro/trn_rl_repo"):
    if os.path.isdir(_p) and _p not in sys.path:
        sys.path.insert(0, _p)

import concourse.bass as bass  # noqa: E402
import concourse.tile as tile  # noqa: E402
from concourse import bacc, mybir  # noqa: E402
from concourse.bass_utils import run_bass_kernel_spmd  # noqa: E402

# Problem constants (hardcoded per contract).
N, CIN, H, W = 32, 3, 224, 224
COUT, KK = 64, 3
NCORES = 8
NPER = N // NCORES          # 4 images per core
WP = W + 2                  # padded width: 226
SR = 56                     # slab rows (output rows materialized per S fill)
NSLAB = H // SR             # 4 slabs per image
CHROWS = 4                  # output rows per matmul chunk
NCOL = (CHROWS // 2) * WP   # 452: matmul moving free size
CPS = SR // CHROWS          # 14 chunks per slab
GROUPS = (4, 4, 4, 2)       # chunks per staging/output-DMA group (sum = CPS)

USE_F32R = True             # float32r: full-rate fp32 matmul (HW rounds internally)

# dest column range [c0, c1) that is filled from x for each dx; the rest is
# zero padding.  dest col c <- x col (c + dx - 1).
ZCOLS = {0: (1, 225), 1: (0, 224), 2: (0, 223)}

F32 = mybir.dt.float32
F32R = mybir.dt.float32r
IDENT = mybir.ActivationFunctionType.Identity


def _emit(ctx, tc, o_ap, x_ap, w_ap, b_ap, repeat=1):
    nc = tc.nc
    mm_dt = F32R if USE_F32R else F32

    wpool = ctx.enter_context(tc.tile_pool(name="wpool", bufs=1))
    spool = ctx.enter_context(tc.tile_pool(name="spool", bufs=1))
    ppool = ctx.enter_context(tc.tile_pool(name="ppool", bufs=8, space="PSUM"))
    stpool = ctx.enter_context(tc.tile_pool(name="stpool", bufs=3))

    # f32r tiles may only be produced by DMA (walrus: "rounded to FP32r"),
    # so zero-fills of f32r tiles go through casting DMAs from this tile.
    zeros_src = wpool.tile([18, 256], F32, name="zeros_src", tag="zeros_src")
    nc.vector.memset(zeros_src[:], 0.0)

    # --- weights: 3 block-diagonal lhsT tiles [18, 128], one per dy ---
    w_src = w_ap.rearrange("co ci dy dx -> dy dx ci co")
    w_tiles = []
    for dy in range(3):
        wt = wpool.tile([18, 128], mm_dt, name=f"w_dy{dy}", tag=f"w_dy{dy}")
        nc.gpsimd.dma_start(wt[:], zeros_src[:, 0:128])
        for g in range(2):
            for dx in range(3):
                p0 = g * 9 + dx * 3
                nc.gpsimd.dma_start(
                    wt[p0 : p0 + 3, g * 64 : (g + 1) * 64], w_src[dy, dx]
                )
        w_tiles.append(wt)

    # --- bias: [128, 1], channels replicated for both g blocks ---
    bias_t = wpool.tile([128, 1], F32, name="bias_t", tag="bias_t")
    b_src = b_ap.rearrange("(c one) -> c one", one=1)
    for g in range(2):
        nc.gpsimd.dma_start(bias_t[g * 64 : (g + 1) * 64, :], b_src)

    # --- shift-tile slabs (2, manually alternated) ---
    FS = SR * WP
    s_tiles = [
        spool.tile([18, FS], mm_dt, name=f"s_slab{i}", tag=f"s_slab{i}") for i in range(2)
    ]

    chunk_no = 0
    slab_no = 0
    for _rep in range(repeat):
        for n in range(NPER):
            for s in range(NSLAB):
                h0 = s * SR
                st_ = s_tiles[slab_no % 2]
                slab_no += 1
                stv = st_.rearrange("p (r c) -> p r c", c=WP)

                # Zero the pad zones first (via casting DMAs from zeros_src —
                # f32r can only be produced by DMA); the fill DMAs then
                # overwrite the valid parts.  Union of pad columns over dx is
                # {0, 223, 224, 225}.
                nc.gpsimd.dma_start(
                    stv[0:18, :, 0:1],
                    zeros_src[:, 0:SR].rearrange("p (r c) -> p r c", c=1),
                )
                nc.gpsimd.dma_start(
                    stv[0:18, :, 223:WP],
                    zeros_src[:, 0 : 3 * SR].rearrange("p (r c) -> p r c", c=3),
                )
                if h0 == 0:
                    nc.gpsimd.dma_start(
                        stv[0:18, 0:1, :],
                        zeros_src[:, 0:WP].rearrange("p (r c) -> p r c", r=1),
                    )
                if h0 + SR == H:
                    nc.gpsimd.dma_start(
                        stv[0:18, SR - 1 : SR, :],
                        zeros_src[:, 0:WP].rearrange("p (r c) -> p r c", r=1),
                    )

                # --- fill S slab: 6 DMAs (g, dx) ---
                for g in range(2):
                    if g == 0:
                        xr0, xr1 = max(0, h0 - 1), h0 + SR - 1
                        rl0 = xr0 - (h0 - 1)
                    else:
                        xr0, xr1 = h0 + 1, min(H, h0 + SR + 1)
                        rl0 = 0
                    nr = xr1 - xr0
                    for dx in range(3):
                        p0 = g * 9 + dx * 3
                        c0, c1 = ZCOLS[dx]
                        xc0 = c0 + dx - 1
                        nc.gpsimd.dma_start(
                            stv[p0 : p0 + 3, rl0 : rl0 + nr, c0:c1],
                            x_ap[n, :, xr0:xr1, xc0 : xc0 + (c1 - c0)],
                        )
                # --- chunks ---
                gh = h0
                for gsz in GROUPS:
                    stage = stpool.tile([128, gsz * NCOL], F32, name="stage", tag="stage")
                    for j in range(gsz):
                        h = gh + j * CHROWS
                        ps = ppool.tile([128, NCOL], F32, name="ps", tag="ps")
                        for dy in range(3):
                            off = (h - h0 + dy) * WP
                            nc.tensor.matmul(
                                ps[:],
                                w_tiles[dy][:],
                                st_[:, off : off + NCOL],
                                start=(dy == 0),
                                stop=(dy == 2),
                            )
                        dst = stage[:, j * NCOL : (j + 1) * NCOL]
                        if chunk_no % 2 == 0:
                            nc.vector.tensor_scalar_add(dst, ps[:], bias_t[:])
                        else:
                            nc.scalar.activation(dst, ps[:], IDENT, bias=bias_t[:], scale=1.0)
                        chunk_no += 1

                    # --- output DMAs (one per (g block, row-within-pair)) ---
                    srcv = stage.rearrange("p (j i w) -> p j i w", i=2, w=WP)
                    dstv = o_ap[n, :, gh : gh + gsz * CHROWS, :].rearrange(
                        "co (j g i) w -> g i co j w", g=2, i=2
                    )
                    for g in range(2):
                        for i in range(2):
                            nc.sync.dma_start(
                                dstv[g, i],
                                srcv[g * 64 : (g + 1) * 64, :, i, 0:W],
                            )
                    gh += gsz * CHROWS


def build_nc(repeat=1):
    nc = bacc.Bacc("TRN2", target_bir_lowering=False, debug=False)
    x_ap = nc.dram_tensor("x", [NPER, CIN, H, W], F32, kind="ExternalInput").ap()
    w_ap = nc.dram_tensor("weight", [COUT, CIN, KK, KK], F32, kind="ExternalInput").ap()
    b_ap = nc.dram_tensor("bias", [COUT], F32, kind="ExternalInput").ap()
    o_ap = nc.dram_tensor("out", [NPER, COUT, H, W], F32, kind="ExternalOutput").ap()

    from contextlib import ExitStack

    with tile.TileContext(nc) as tc:
        with ExitStack() as ctx:
            _emit(ctx, tc, o_ap, x_ap, w_ap, b_ap, repeat=repeat)
    nc.compile()
    return nc


_NC_CACHE = {}


def _get_nc(repeat=1):
    if repeat not in _NC_CACHE:
        _NC_CACHE[repeat] = build_nc(repeat=repeat)
    return _NC_CACHE[repeat]


def run_cores(x, weight, bias, repeat=1):
    x = np.ascontiguousarray(np.asarray(x), dtype=np.float32)
    weight = np.ascontiguousarray(np.asarray(weight), dtype=np.float32)
    bias = np.ascontiguousarray(np.asarray(bias), dtype=np.float32)
    nc = _get_nc(repeat=repeat)
    in_maps = [
        {"x": x[c * NPER : (c + 1) * NPER], "weight": weight, "bias": bias}
        for c in range(NCORES)
    ]
    res = run_bass_kernel_spmd(nc, in_maps, list(range(NCORES))).results
    return np.concatenate([res[c]["out"] for c in range(NCORES)], axis=0)


def kernel(x, weight, bias):
    return run_cores(x, weight, bias, repeat=1)
